# revision 43
# baseline (speedup 1.0000x reference)
"""Involution (B=4, C=256, H=W=56, K=7, G=16, reduction=4) on 8 trn2 NeuronCores.

Sharding: 8 shards = (batch b in 0..3) x (h-half in 0..1); each core computes
its [256, 28, 56] output slab.

"(group, col-block)-major" design. Per core, partition p = g*8 + cb encodes
(group g in 0..15, 7-output-col block cb in 0..7) -- all 128 partitions, both
channel halves in one 49-tap loop. Pixels are COLUMN-major throughout. The
per-pixel kernel w stays COMPACT (no 16x channel broadcast): the DVE multiply
reads it through a stride-0 free-dim AP, replicating each (g,cb) row across
the 16 channels of its group for free, and runs in DVE 2x mode (bf16, packed
inner runs of 28).

Pipeline (per core):
  1. inputs: one fat DMA per tensor on sync/scalar HWDGE only (each DMA has
     a ~30ns/descriptor latency floor; gpsimd SWDGE is ~10us/128-desc). The
     row-shifted slab copy is built on-chip (plain SBUF->SBUF DMA).
  2. stage1 (PE+Act): t_ext = [relu(bn(W1 @ x)); ones]  [65, 1568] bf16,
     column-chunked behind the two xc half-DMAs, 3 PSUM bufs.
  3. w-gen (PE): septets 0-1 through a 4-bank PSUM scratch (released before
     the 7-bank accumulator opens), evacs split DVE||Act; septets 2-6
     bounce through a single dedicated PSUM bank, interleaved into row
     (kt-2)'s accumulation stream (engines execute in order).
  4. rearrange (DMA): per-kp partition-scatter stores into DRAM; septet 0
     reloads per-tap (each tap gates on its own store+load chain), later
     septets reload as one contiguous [128, 1372] descriptor set.
     (SBUF-dst partition-split DMA views scatter to wrong addresses --
     measured -- so a direct SBUF->SBUF rearrange is not an option.)
  5. main loop: 49 per-tap DVE multiplies (the TENSOR3D ISA pattern caps
     tensor_tensor at 3 free dims, so taps cannot be merged). Odd rows read
     the row-shifted slab so every inner run stays 4B-aligned (keeps DVE 2x
     mode penalty-free; all 49 taps run ~1782ns, gap-free).
  6. tail: PSUM->bf16 evac split DVE||Act in parallel halves (PSUM fp32
     reads are 1x on both), two parallel output DMAs.

Engines: DVE is the critical path (the 49 taps of elementwise muls are
irreducible: PE cannot contract per-pixel weights, Act only scales
per-partition, GpSimd shares the 2nd DVE SBUF port). PE trails with identity
matmul accumulation into a 7-bank PSUM accumulator. All steady-state DMAs go
through HWDGE (sync/scalar); gpsimd only issues DMAs before the first tap.
"""

import numpy as np
import ml_dtypes
from contextlib import ExitStack

import concourse.bass as bass
import concourse.bacc as bacc
import concourse.tile as tile
from concourse import mybir
from concourse.bass_utils import run_bass_kernel_spmd

BF16 = ml_dtypes.bfloat16

B, C, H, W = 4, 256, 56, 56
KK, G, PAD = 7, 16, 3
Cr = 64
EPS = 1e-5
HH = H // 2              # 28 rows per h-half shard
PH, PW = HH + 2 * PAD, W + 2 * PAD   # 34, 62 padded slab dims
NPIX = HH * W            # 1568 output pixels per shard
NCORES = 8

CB = 7                   # output cols per block
CIN = CB + 2 * PAD       # input cols held per partition slab (13)
NBLK = W // CB           # 8 col-blocks
NP = G * NBLK            # 128 partitions used
PIXB = CB * HH           # 196 pixels per block
FREE = 16 * PIXB         # 3136 elements per partition per tap
NWP = 112                # w-gen partitions (g*7 + kp)
NTAP = KK * KK           # 49
XFREE = G * CIN * PH     # 7072 elements per partition of the x slab

_CACHE = {}

TRACE = False
LAST_RESULT = None


def _build_nc():
    nc = bacc.Bacc("TRN2", target_bir_lowering=False, debug=False,
                   num_devices=NCORES)

    f32 = mybir.dt.float32
    bf16 = mybir.dt.bfloat16

    xc_d = nc.declare_dram_parameter("xc", [128, 2, W, HH], bf16, isOutput=False)
    xr_d = nc.declare_dram_parameter("xr", [NP, G, CIN, PH], bf16, isOutput=False)
    w1t_d = nc.declare_dram_parameter("w1t", [128, 2, Cr], bf16, isOutput=False)
    b1p_d = nc.declare_dram_parameter("b1p", [Cr, 1], f32, isOutput=False)
    w2t_d = nc.declare_dram_parameter("w2t", [Cr + 1, KK, NWP], bf16, isOutput=False)
    ident_d = nc.declare_dram_parameter("ident", [NP, NP], bf16, isOutput=False)
    out_d = nc.declare_dram_parameter("out", [NP, G, CB, HH], bf16, isOutput=True)

    # per-septet bounce buffer: [kt][(g,cb) row, kp, block-pixels]; per-tap
    # and per-septet reloads are contiguous runs per partition.
    wdram = nc.dram_tensor("wshuf", [KK, NP, KK, PIXB], bf16)

    with tile.TileContext(nc) as tc, ExitStack() as ctx:
        const = ctx.enter_context(tc.tile_pool(name="const", bufs=1))
        xpool = ctx.enter_context(tc.tile_pool(name="x", bufs=1))
        tpool = ctx.enter_context(tc.tile_pool(name="t", bufs=1))
        wcmpp = ctx.enter_context(tc.tile_pool(name="wc", bufs=2))
        wrtp = ctx.enter_context(tc.tile_pool(name="wrt", bufs=1))

        # ---- input DMAs: few, big, early, spread over the 5 engine queues
        xc_sb = xpool.tile([128, 2, W, HH], bf16)
        xr_sb = xpool.tile([NP, G, CIN, PH], bf16)
        xr2_sb = xpool.tile([NP, G, CIN, PH], bf16)   # row-shifted copy
        w1t_sb = const.tile([128, 2, Cr], bf16)
        b1p_sb = const.tile([Cr, 1], f32)
        w2t_sb = const.tile([Cr + 1, KK, NWP], bf16)
        ident_sb = const.tile([NP, NP], bf16)

        # Input DMAs on sync+scalar ONLY (gpsimd dma_start is SWDGE --
        # ~80ns/descriptor software generation makes a 128-descriptor load
        # take ~10us). Every DMA here has a ~30ns/descriptor latency floor,
        # so each input goes as ONE fat DMA (128 descriptors max).
        # Each engine's DMA ring processes its queue IN ORDER, so the two
        # stage1-critical xc halves both go FIRST on sync's ring (a half
        # queued behind w1t on scalar was measured landing at 19us); the
        # big x-slab is split across both rings behind the critical items.
        nc.sync.dma_start(xc_sb[:, :, 0:28, :], xc_d[:, :, 0:28, :])
        nc.sync.dma_start(xc_sb[:, :, 28:56, :], xc_d[:, :, 28:56, :])
        nc.scalar.dma_start(w1t_sb[:], w1t_d[:])
        nc.scalar.dma_start(b1p_sb[:], b1p_d[:])
        nc.scalar.dma_start(w2t_sb[:], w2t_d[:])
        xr_flat = xr_sb[:].rearrange("p a c r -> p (a c r)")
        xrd_flat = xr_d[:].rearrange("p a c r -> p (a c r)")
        nc.sync.dma_start(xr_flat[:, 0:XFREE // 2], xrd_flat[:, 0:XFREE // 2])
        nc.scalar.dma_start(xr_flat[:, XFREE // 2:], xrd_flat[:, XFREE // 2:])
        nc.sync.dma_start(ident_sb[:], ident_d[:])
        xr2_flat = xr2_sb[:].rearrange("p a c r -> p (a c r)")

        # ---- stage 1 + w-gen kt=0 ----
        t_ext = tpool.tile([Cr + 1, NPIX], bf16)
        nc.vector.memset(t_ext[Cr:Cr + 1, :], 1.0)

        w_cmp = [wcmpp.tile([NWP, NPIX], bf16, name=f"wc{kt}", tag="wc")
                 for kt in range(KK)]
        w_rT = wrtp.tile([NP, NTAP, CB, HH], bf16)

        # w rearrange goes through a DRAM bounce: SBUF-dst partition-split
        # DMA views scatter to wrong addresses (measured garbage), so
        # SBUF->SBUF is not an option. wdram layout [kt][NP, kp, 196] makes
        # every reload contiguous per partition.
        def store_septet(kt):
            for kp in range(KK):
                src = w_cmp[kt][kp:NWP:KK].rearrange("g (cb x) -> g cb x",
                                                     cb=NBLK)
                dst = wdram[kt][:, kp, :].rearrange("(g cb) x -> g cb x",
                                                    cb=NBLK)
                eng = nc.scalar if kp % 2 == 0 else nc.sync
                eng.dma_start(dst, src)

        def load_septet(kt):
            dst = w_rT[:, kt * KK:(kt + 1) * KK].rearrange(
                "p k r c -> p (k r c)")
            src = wdram[kt].rearrange("p k x -> p (k x)")
            nc.sync.dma_start(dst, src)

        def load_tap(k, eng):
            kt, kp = k // KK, k % KK
            eng.dma_start(w_rT[:, k].rearrange("p r c -> p (r c)"),
                          wdram[kt][:, kp, :])

        def emit_septet0_rearrange():
            # stores: scalar takes even kp (right behind its evac), sync
            # odd; per-tap loads all on sync so tap k waits only store k.
            store_septet(0)
            for kp in range(KK):
                load_tap(kp, nc.sync)

        pwp = ctx.enter_context(tc.tile_pool(name="pw", bufs=1,
                                             space=bass.MemorySpace.PSUM))
        NCW = 14
        WCH = NCW * HH      # 392
        with tc.tile_pool(name="pwbig", bufs=1,
                          space=bass.MemorySpace.PSUM) as pwbigp, \
             tc.tile_pool(name="psum_t", bufs=3,
                          space=bass.MemorySpace.PSUM) as psum_t:
            for q in range(W // NCW):
                pt = psum_t.tile([Cr, WCH], f32)
                for ch in range(2):
                    rhs = xc_sb[:, ch, q * NCW:(q + 1) * NCW, :]
                    nc.tensor.matmul(pt[:], w1t_sb[:, ch, :], rhs,
                                     start=(ch == 0), stop=(ch == 1))
                nc.scalar.activation(
                    t_ext[0:Cr, q * WCH:(q + 1) * WCH],
                    pt[:], mybir.ActivationFunctionType.Relu,
                    bias=b1p_sb[:], scale=1.0)
            # w-gen kt=0 and kt=1 both run through the 4-bank scratch with
            # one big evac each (emitted after all stage1 MMs so the
            # in-order PE never waits on a t_ext evac that hasn't drained)
            for kt in range(2):
                pwbig = pwbigp.tile([NWP, 4, 512], f32, tag="pwbig")
                for q in range(4):
                    nc.tensor.matmul(pwbig[:, q, 0:WCH], w2t_sb[:, kt, :],
                                     t_ext[:, q * WCH:(q + 1) * WCH],
                                     start=True, stop=True)
                # evac split across DVE+Act in parallel (both read PSUM
                # fp32 at 1x); the DVE half is safely pre-TT0 in-order.
                half = w_cmp[kt][:].rearrange("p (a x) -> p a x", a=4)
                nc.vector.tensor_copy(half[:, 0:2], pwbig[:, 0:2, 0:WCH])
                nc.scalar.copy(half[:, 2:4], pwbig[:, 2:4, 0:WCH])
                if kt == 0:
                    emit_septet0_rearrange()
                    # row-shifted slab copy (needed only from row 1): emit
                    # behind the septet-0 loads so it never blocks them
                    nc.sync.dma_start(xr2_flat[:, 0:XFREE - 1],
                                      xr_flat[:, 1:XFREE])

        # ---- main PSUM accumulator (7 banks; bank 8 stays with pwp) ----
        ACHUNKS = [(0, 512), (512, 512), (1024, 512), (1536, 512),
                   (2048, 512), (2560, 512), (3072, 64)]
        accp = ctx.enter_context(tc.tile_pool(name="psum_acc", bufs=1,
                                              space=bass.MemorySpace.PSUM))
        accpad = accp.tile([NP, 3584], f32)
        acc = accpad[:, 0:FREE]

        store_septet(1)
        load_septet(1)

        # ---- w-gen kt>=2: single PSUM bank, chunk-serialized, interleaved
        # into row (kt-2)'s accumulation stream so the in-order PE reaches
        # each chunk well after its predecessor's evac has drained.
        def wgen_chunk(kt, cch):
            pw = pwp.tile([NWP, 512], f32, tag="pw")
            nc.tensor.matmul(pw[:, 0:WCH], w2t_sb[:, kt, :],
                             t_ext[:, cch * WCH:(cch + 1) * WCH],
                             start=True, stop=True)
            nc.scalar.copy(w_cmp[kt][:, cch * WCH:(cch + 1) * WCH],
                           pw[:, 0:WCH])

        # ---- main loop: 49 per-tap DVE multiplies (ISA caps TT at 3 free
        # dims, so taps cannot be merged); odd rows read the row-shifted
        # slab so every inner run starts 4B-aligned and DVE 2x mode never
        # pays the misaligned-run peel.
        prodp = ctx.enter_context(tc.tile_pool(name="prod", bufs=4))
        outp = ctx.enter_context(tc.tile_pool(name="outp", bufs=1))

        for i in range(KK):
            kt_n = i + 2     # septet generated while row i accumulates
            xbase, off = (xr_sb, i) if i % 2 == 0 else (xr2_sb, i - 1)
            for j in range(KK):
                k = i * KK + j
                xwin = xbase[:, :, j:j + CB, off:off + HH]
                wtap = w_rT[:, k].unsqueeze(1).broadcast_to([NP, G, CB, HH])
                pr = prodp.tile([NP, G, CB, HH], bf16, tag="prod")
                nc.vector.tensor_mul(pr[:], xwin, wtap)
                prf = pr[:].rearrange("p a r c -> p (a r c)")
                for (o, n) in ACHUNKS:
                    nc.tensor.matmul(acc[:, o:o + n], ident_sb[:],
                                     prf[:, o:o + n],
                                     start=(k == 0), stop=(k == NTAP - 1))
                if kt_n < KK:
                    if j < 4:
                        wgen_chunk(kt_n, j)
                    elif j == 4:
                        store_septet(kt_n)
                    elif j == 5:
                        load_septet(kt_n)

        # ---- tail: PSUM fp32 reads run at 1x on both DVE and Act, so the
        # evac is split across the two engines in parallel, each half's
        # output DMA issued as soon as its evac lands.
        of = outp.tile([NP, G, CB, HH], bf16)
        off_ = of[:].rearrange("p a r c -> p (a r c)")
        outf = out_d[:].rearrange("p a r c -> p (a r c)")
        HF = FREE // 2
        nc.vector.tensor_copy(off_[:, 0:HF], acc[:, 0:HF])
        nc.scalar.copy(off_[:, HF:FREE], acc[:, HF:FREE])
        nc.sync.dma_start(outf[:, 0:HF], off_[:, 0:HF])
        nc.scalar.dma_start(outf[:, HF:FREE], off_[:, HF:FREE])

    nc.compile()
    return nc


def _prep_host_inputs(inputs, W1, b1, gamma, beta, mean, var, W2, b2):
    """Fold BN into W1/b1; build per-core rearranged inputs and W2 tiles."""
    scale = gamma / np.sqrt(var + EPS)
    shift = beta - mean * scale
    W1p = W1 * scale[:, None]
    b1p = (b1 * scale + shift).astype(np.float32).reshape(Cr, 1)
    w1t = np.ascontiguousarray(
        W1p.T.reshape(2, 128, Cr).transpose(1, 0, 2)).astype(BF16)

    # w2t[o, kt, g*7+kp] = W2e[g*49 + kt*7 + kp, o]
    W2e = np.concatenate([W2, b2[:, None]], axis=1)      # [784, 65]
    p_idx = np.arange(NWP)
    kt_idx = np.arange(KK)
    rows = (p_idx[None, :] // KK) * NTAP + kt_idx[:, None] * KK \
        + (p_idx[None, :] % KK)                          # [7, 112]
    w2t = np.ascontiguousarray(W2e[rows].transpose(2, 0, 1)).astype(BF16)

    ident = np.eye(NP, dtype=np.float32).astype(BF16)

    xcs, xrs = [], []
    for core in range(NCORES):
        bt, hf = core // 2, core % 2
        slab = np.zeros((C, PH, PW), np.float32)
        r0 = hf * HH - PAD
        r1 = r0 + PH
        v0, v1 = max(r0, 0), min(r1, H)
        slab[:, v0 - r0:v1 - r0, PAD:PAD + W] = inputs[bt, :, v0:v1, :]
        xcs.append(np.ascontiguousarray(
            slab[:, PAD:PAD + HH, PAD:PAD + W].reshape(2, 128, HH, W)
            .transpose(1, 0, 3, 2)).astype(BF16))
        xg = slab.reshape(G, 16, PH, PW)
        xr = np.stack([xg[:, :, :, CB * cb:CB * cb + CIN]
                       for cb in range(NBLK)], axis=1)   # [16, 8, 16, 34, 13]
        xr = xr.transpose(0, 1, 2, 4, 3)                 # [16, 8, 16, 13, 34]
        xrs.append(np.ascontiguousarray(
            xr.reshape(NP, 16, CIN, PH)).astype(BF16))
    return xcs, xrs, w1t, b1p, w2t, ident


def kernel(inputs, W1, b1, gamma, beta, mean, var, W2, b2):
    global LAST_RESULT
    inputs = np.asarray(inputs, np.float32)
    if "nc" not in _CACHE:
        _CACHE["nc"] = _build_nc()
    nc = _CACHE["nc"]

    xcs, xrs, w1t, b1p, w2t, ident = _prep_host_inputs(
        inputs, np.asarray(W1, np.float32), np.asarray(b1, np.float32),
        np.asarray(gamma, np.float32), np.asarray(beta, np.float32),
        np.asarray(mean, np.float32), np.asarray(var, np.float32),
        np.asarray(W2, np.float32), np.asarray(b2, np.float32))

    in_maps = [{"xc": xcs[core], "xr": xrs[core], "w1t": w1t, "b1p": b1p,
                "w2t": w2t, "ident": ident} for core in range(NCORES)]
    res = run_bass_kernel_spmd(nc, in_maps, list(range(NCORES)), trace=TRACE)
    LAST_RESULT = res

    out = np.empty((B, C, H, W), np.float32)
    for core in range(NCORES):
        bt, hf = core // 2, core % 2
        r = np.asarray(res.results[core]["out"], np.float32)
        slab = r.reshape(G, NBLK, 16, CB, HH).transpose(0, 2, 4, 1, 3)
        out[bt, :, hf * HH:(hf + 1) * HH, :] = slab.reshape(C, HH, W)
    return out
